# revision 8
# baseline (speedup 1.0000x reference)
"""LocalFeatureAggregation Trainium2 Bass kernel (v2: transfer-optimized).

Reference computation (per batch b, point n):
  t[n,k,:]   = LeakyReLU_0.1(geom[n,k,:] @ w.T + b)          # [N,K,D], D=64
  fn[n,k,:]  = features[idx[n,k], :]                          # [N,K,C], C=64
  out[n,:]   = concat(mean_k t, mean_k fn)                    # [N, 128]

The end-to-end time on this deployment is dominated by host<->device
transfer through the axon tunnel (~65 MB/s; outputs cost double because
PJRT uploads donated zero buffers). So v2 minimizes bytes moved:
  - all float payloads cross the tunnel as bf16, indices as int16
    (N=16384 < 2^15); rel tolerance is 2e-2, bf16 adds ~0.5% worst case
  - no data duplication across cores: core = (b = core//2, h = core%2)
      t-side: points h*8192..+8192 of batch b (geom pre-transposed on
              host into the matmul-ready layout)
      f-side: ALL 16384 points of batch b, feature channels 32h..32h+32
  - outputs download as bf16 in device-native layout; host reindexes

Per-core device dataflow:
  t-side: bt [128, 4096] bf16 uploaded with partition p = 64*n2+4*k+f,
    col = 128*t + r (point n = 256*t + 2*r + n2). Per (group tg, k): one
    matmul with host-built block-stationary S_k bf16 -> PSUM [128=(n2,d),512],
    ACT Prelu in-place on PSUM (scale 1/16, bias b/16), DVE accumulates
    PSUM into f32 acc. ACT copies acc -> bf16, one DMA store.
    Every cross-engine handoff is single-producer: each instruction
    needs <=1 sem wait (walrus limit).
  f-side: per-row indirect DMA gathers from fh (DRAM, [16384,32] bf16),
    one 64B row per partition per op (row n = 128q + p), CCE
    accumulate=add folds the 16-neighbor sum into the DMA; one ACT
    scale by 1/16 -> store in [p, q, c] layout.
"""

import sys

sys.path.insert(0, "/opt/trn_rl_repo")

import numpy as np
import ml_dtypes

import concourse.bass as bass
import concourse.tile as tile
from concourse import mybir
from concourse.bass_utils import run_bass_kernel_spmd

P = 128
B, N, K, C, D = 4, 16384, 16, 64, 64
NH = N // 2            # t-side points per core
CH = C // 2            # f-side channels per core
Q = N // P             # f-side point groups (128)
T = NH * K * 4 // (P * P)  # 32 geom col-groups of 128
G = 8                  # matmul groups
W = T * P // G         # moving free dim per group (512)
F32 = mybir.dt.float32
BF = mybir.dt.bfloat16
I32 = mybir.dt.int32
I16 = mybir.dt.int16
BF_NP = np.dtype(ml_dtypes.bfloat16)

_CACHE = {}


class _SplitDrainTC(tile.TileContext):
    """TileContext whose tail drain splits its sem waits across multiple
    single-wait drain instructions (walrus accepts one sync-wait per
    instruction on this path)."""

    def _drain_and_barrier(self, tick_clock, wait_clock):
        from concourse.vector_clock import ScopedClock

        drain_inst = self.nc.sync.drain()
        wait_clock.add_sem_waits(
            drain_inst.ins, ScopedClock({None: tick_clock.global_clock})
        )
        inst = drain_inst.ins
        si = inst.sync_info
        waits = list(si.on_wait) if si else []
        if len(waits) > 1:
            si.on_wait = waits[:1]
            for w in waits[1:]:
                d2 = self.nc.sync.drain().ins
                if d2.sync_info is None:
                    d2.sync_info = mybir.SyncInfo(on_wait=[w], on_update=[])
                else:
                    d2.sync_info.on_wait = [w]
        self.nc.all_engine_barrier()
        popped = self.nc._tile_sem_poison_stack.pop()
        assert popped is self._sem_poison
        self.nc.clear_and_free_semaphores(list(self.sems.allocated().values()))
        self.nc.all_engine_barrier()


def _build_program(_variant="full"):
    nc = bass.Bass(
        "TRN2",
        target_bir_lowering=False,
        debug=False,
        enable_asserts=False,
        num_devices=8,
    )
    fh = nc.dram_tensor("fh", [N, CH], BF, kind="ExternalInput")
    gt = nc.dram_tensor("gt", [P, T * P], BF, kind="ExternalInput")
    ixd = nc.dram_tensor("ix", [P, Q * K], I16, kind="ExternalInput")
    sd = nc.dram_tensor("s", [P, K * P], BF, kind="ExternalInput")
    bd = nc.dram_tensor("bias", [P, 2], F32, kind="ExternalInput")
    ot_d = nc.dram_tensor("ot", [P, T * P], BF, kind="ExternalOutput")
    of_d = nc.dram_tensor("of", [P, Q * CH], BF, kind="ExternalOutput")

    from contextlib import ExitStack

    with _SplitDrainTC(nc) as tc, ExitStack() as ctx:
        const = ctx.enter_context(tc.tile_pool(name="const", bufs=1))
        big = ctx.enter_context(tc.tile_pool(name="big", bufs=1))
        tmp = ctx.enter_context(tc.tile_pool(name="tmp", bufs=2))
        ps1 = ctx.enter_context(tc.tile_pool(name="ps1", bufs=3, space="PSUM"))
        psw = ctx.enter_context(tc.tile_pool(name="psw", bufs=1, space="PSUM"))

        s_sb = const.tile([P, K * P], BF)
        nc.sync.dma_start(s_sb[:], sd.ap())
        b_sb = const.tile([P, 2], F32)
        nc.sync.dma_start(b_sb[:], bd.ap())
        ix16 = const.tile([P, Q * K], I16)
        nc.sync.dma_start(ix16[:], ixd.ap())
        bt = big.tile([P, T * P], BF)
        nc.sync.dma_start(bt[:], gt.ap())

        # Index upcast (the SWDGE index buffer wants i32); also absorbs the
        # ix DMA lane into the DVE clock.
        ix32 = const.tile([P, Q * K], I32)
        nc.vector.tensor_copy(ix32[:], ix16[:])

        # Warm-up observer ops: absorb each input-load DMA lane into the
        # engine that will consume that tensor, keeping every later
        # instruction at <=1 sync wait (walrus limit).
        warm_sb = tmp.tile([P, 1], F32)
        nc.scalar.activation(
            warm_sb[:], b_sb[:, 0:1], mybir.ActivationFunctionType.Copy,
            bias=0.0, scale=1.0,
        )
        wp = psw.tile([P, 2], F32)
        nc.tensor.matmul(
            out=wp[:, 0:1], lhsT=s_sb[:, 0:P], rhs=s_sb[:, 0:1],
            start=True, stop=True,
        )
        nc.tensor.matmul(
            out=wp[:, 1:2], lhsT=bt[:, 0:P], rhs=bt[:, 0:1],
            start=True, stop=True,
        )

        # -------- f-side: per-row indirect gathers, K-mean in the DMA ----
        # Each op gathers one 64B feature row per partition (row n =
        # 128*q + p) and CCE-accumulates over the 16 neighbors of each
        # point; one ACT pass applies the 1/K scale.
        if _variant == "diaggather":
            # Raw dump of the multi-index gather staging for the first 8
            # point groups: recovers the HW descriptor ordering.
            stg = ctx.enter_context(tc.tile_pool(name="stg", bufs=4))
            for q in range(8):
                st = stg.tile([P, K * CH], BF, tag="st")
                nc.gpsimd.indirect_dma_start(
                    out=st[:],
                    out_offset=None,
                    in_=fh.ap(),
                    in_offset=bass.IndirectOffsetOnAxis(
                        ap=ix32[:, bass.ts(q, K)], axis=0
                    ),
                )
                nc.sync.dma_start(of_d.ap()[:, bass.ts(q, K * CH)], st[:])
        elif _variant == "biggather":
            # One gather op per point group: offsets [128, K] -> staging
            # [p, (k, c)], then one DVE reduce over k (strided inner axis)
            # into f32 facc. 16x fewer SWDGE ops, same descriptor count.
            faccf = big.tile([P, Q * CH], F32)
            stg = ctx.enter_context(tc.tile_pool(name="stg", bufs=4))
            for q in range(Q):
                st = stg.tile([P, K * CH], BF, tag="st")
                nc.gpsimd.indirect_dma_start(
                    out=st[:],
                    out_offset=None,
                    in_=fh.ap(),
                    in_offset=bass.IndirectOffsetOnAxis(
                        ap=ix32[:, bass.ts(q, K)], axis=0
                    ),
                )
                nc.vector.tensor_reduce(
                    faccf[:, bass.ts(q, CH)],
                    st[:].rearrange("p (k c) -> p c k", k=K),
                    mybir.AxisListType.X,
                    mybir.AluOpType.add,
                )
            facc = big.tile([P, Q * CH], BF)
            nc.scalar.activation(
                facc[:], faccf[:], mybir.ActivationFunctionType.Copy,
                bias=0.0, scale=1.0 / K,
            )
            nc.sync.dma_start(of_d.ap(), facc[:])
        else:
            facc = big.tile([P, Q * CH], BF)  # [p, (q, c)]; n = 128*q + p
            if _variant == "nogather":
                nc.vector.memset(facc[:], 0.0)
            else:
                for q in range(Q):
                    for k in range(K):
                        nc.gpsimd.indirect_dma_start(
                            out=facc[:, bass.ts(q, CH)],
                            out_offset=None,
                            in_=fh.ap(),
                            in_offset=bass.IndirectOffsetOnAxis(
                                ap=ix32[:, q * K + k : q * K + k + 1], axis=0
                            ),
                            compute_op=(
                                mybir.AluOpType.add if k else mybir.AluOpType.bypass
                            ),
                        )
                nc.scalar.activation(
                    facc[:], facc[:], mybir.ActivationFunctionType.Copy,
                    bias=0.0, scale=1.0 / K,
                )
            nc.sync.dma_start(of_d.ap(), facc[:])

        # ---------------- t-side ----------------------------------------
        # MM1 per k (block stationary) -> ACT Prelu in place on PSUM ->
        # DVE accumulate over k into f32 acc -> ACT downcast -> store.
        acc = big.tile([P, T * P], F32)    # [(n2,d), (t, r)]
        ot_sb = big.tile([P, T * P], BF)
        for tg in range(G):
            for j in range(K):
                ps = ps1.tile([P, W], F32, tag="ps")
                nc.tensor.matmul(
                    out=ps[:],
                    lhsT=s_sb[:, bass.ts(j, P)],
                    rhs=bt[:, bass.ts(tg, W)],
                    start=True,
                    stop=True,
                )
                nc.scalar.activation(
                    ps[:],
                    ps[:],
                    mybir.ActivationFunctionType.Prelu,
                    bias=b_sb[:, 0:1],
                    scale=1.0 / K,
                    alpha=b_sb[:, 1:2],
                )
                if j == 0:
                    nc.vector.tensor_copy(acc[:, bass.ts(tg, W)], ps[:])
                else:
                    nc.vector.tensor_add(
                        acc[:, bass.ts(tg, W)], acc[:, bass.ts(tg, W)], ps[:]
                    )
            nc.scalar.activation(
                ot_sb[:, bass.ts(tg, W)],
                acc[:, bass.ts(tg, W)],
                mybir.ActivationFunctionType.Copy,
                bias=0.0,
                scale=1.0,
            )
        nc.sync.dma_start(ot_d.ap(), ot_sb[:])

    # Walrus accepts at most one sync-wait per instruction. Tile sometimes
    # emits an extra *same-engine* wait (engine completion sem); on the
    # in-order compute engines those are trivially satisfied by queue order,
    # so strip them.
    _ENGINE_SEM = {
        mybir.EngineType.PE: "PE_",
        mybir.EngineType.Activation: "Activation_",
        mybir.EngineType.DVE: "DVE_",
    }
    for inst in nc.inst_map.values():
        si = inst.sync_info
        if si is None or len(si.on_wait) <= 1:
            continue
        pref = _ENGINE_SEM.get(inst.engine)
        if pref is None:
            continue
        keep = [w for w in si.on_wait if not w.ant_name.startswith(pref)]
        if len(keep) < len(si.on_wait) and len(keep) <= 1:
            si.on_wait = keep

    # The Prelu ACTs wait on both their producing matmul (PE) and the PSUM
    # slot's previous DVE consumer. The matmul itself already waited on that
    # same DVE threshold before executing, so the ACT's PE wait implies the
    # DVE wait transitively. Verify coverage and strip.
    last_mm_dve: dict[str, int] = {}
    for inst in nc.inst_map.values():
        si = inst.sync_info
        if isinstance(inst, mybir.InstMatmult):
            for w in si.on_wait if si else []:
                if w.ant_name.startswith("DVE_"):
                    last_mm_dve[w.ant_name] = max(
                        w.wait_value, last_mm_dve.get(w.ant_name, 0)
                    )
        if (
            inst.engine == mybir.EngineType.Activation
            and si is not None
            and len(si.on_wait) > 1
        ):
            pe = [w for w in si.on_wait if w.ant_name.startswith("PE_")]
            dve = [w for w in si.on_wait if w.ant_name.startswith("DVE_")]
            if (
                len(pe) == 1
                and len(pe) + len(dve) == len(si.on_wait)
                and all(last_mm_dve.get(w.ant_name, -1) >= w.wait_value for w in dve)
            ):
                si.on_wait = pe

    # The chained accumulating gathers issue on one SWDGE FIFO and each
    # partition's descriptors drain on a fixed SDMA engine in order, so
    # cross-lane WAW completion waits between them are redundant.
    for inst in nc.inst_map.values():
        if not isinstance(inst, mybir.InstDMACopy):
            continue
        if getattr(inst, "queue", "") != "qPoolDynamic":
            continue
        si = inst.sync_info
        if si is None or len(si.on_wait) <= 1:
            continue
        non_sw = [w for w in si.on_wait if not w.ant_name.startswith("DMASW")]
        sw = [w for w in si.on_wait if w.ant_name.startswith("DMASW")]
        keep = non_sw if non_sw else sw[:1]
        if len(keep) == 1:
            si.on_wait = keep

    # Any instruction still waiting several SWDGE lanes: the gathers issue
    # on one FIFO and each SDMA engine drains its ring in order, so the
    # last lane's completion implies the earlier ones. Keep the last.
    for inst in nc.inst_map.values():
        si = inst.sync_info
        if si is None or len(si.on_wait) <= 1:
            continue
        sw = [w for w in si.on_wait if w.ant_name.startswith("DMASW")]
        if len(sw) == len(si.on_wait):
            si.on_wait = sw[-1:]

    # The output stores write disjoint DRAM; keep only the compute-producer
    # wait if Tile added a false extra one.
    for inst in nc.inst_map.values():
        if not isinstance(inst, mybir.InstDMACopy):
            continue
        si = inst.sync_info
        if si is None or len(si.on_wait) <= 1:
            continue
        memrefs = {getattr(a, "memref", "") for a in inst.outs}
        if memrefs <= {"ot", "of"}:
            act = [w for w in si.on_wait if w.ant_name.startswith("Activation_")]
            sw = [w for w in si.on_wait if w.ant_name.startswith("DMASW")]
            if len(act) == 1:
                si.on_wait = act
            elif len(sw) == 1:
                si.on_wait = sw
    return nc


def _host_inputs(features, geom, w, bvec, nbr):
    """Build the 8 per-core input dicts (pure layout prep, bf16/i16)."""
    S = np.zeros((P, K, P), np.float32)
    wT = np.ascontiguousarray(w.T)  # [4, 64]
    for j in range(K):
        for n2 in range(2):
            S[64 * n2 + 4 * j : 64 * n2 + 4 * j + 4, j, 64 * n2 : 64 * n2 + 64] = wT
    s_host = np.ascontiguousarray(S.reshape(P, K * P)).astype(BF_NP)
    bias_host = np.zeros((P, 2), np.float32)
    bias_host[:, 0] = np.tile(bvec / K, 2)
    bias_host[:, 1] = 0.1

    in_maps = []
    for core in range(8):
        b, h = divmod(core, 2)
        n0 = h * NH
        # bt layout: partition 64*n2 + 4*k + f, col 128*t + r;
        # point n = n0 + 256*t + 2*r + n2
        gt_host = np.ascontiguousarray(
            geom[b, n0 : n0 + NH]
            .reshape(T, P, 2, K, 4)
            .transpose(2, 3, 4, 0, 1)
            .reshape(P, T * P)
        ).astype(BF_NP)
        fh_host = np.ascontiguousarray(
            features[b][:, CH * h : CH * h + CH]
        ).astype(BF_NP)
        # ix[p, q*K+k] = nbr[b][128*q + p, k]
        ix_host = np.ascontiguousarray(
            nbr[b].reshape(Q, P, K).transpose(1, 0, 2).reshape(P, Q * K)
        ).astype(np.int16)
        in_maps.append(
            {
                "fh": fh_host,
                "gt": gt_host,
                "ix": ix_host,
                "s": s_host,
                "bias": bias_host,
            }
        )
    return in_maps


def kernel(**inputs):
    features = np.asarray(inputs["features"], np.float32)
    geom = np.asarray(inputs["geom_features"], np.float32)
    w = np.asarray(inputs["w"], np.float32)
    bvec = np.asarray(inputs["b"], np.float32)
    nbr = np.asarray(inputs["neighbor_indices"])

    if "nc" not in _CACHE:
        _CACHE["nc"] = _build_program()
    nc = _CACHE["nc"]

    in_maps = _host_inputs(features, geom, w, bvec, nbr)
    res = run_bass_kernel_spmd(nc, in_maps, list(range(8)))

    out = np.empty((B, N, 2 * D), np.float32)
    for core in range(8):
        b, h = divmod(core, 2)
        n0 = h * NH
        ot = np.asarray(res.results[core]["ot"]).astype(np.float32)  # [128, T*P]
        out[b, n0 : n0 + NH, :D] = (
            ot.reshape(2, D, T, P).transpose(2, 3, 0, 1).reshape(NH, D)
        )
        of = np.asarray(res.results[core]["of"]).astype(np.float32)  # [128, Q*CH]
        out[b, :, D + CH * h : D + CH * h + CH] = (
            of.reshape(P, Q, CH).transpose(1, 0, 2).reshape(N, CH)
        )
    return out


# revision 9
# speedup vs baseline: 4.1357x; 4.1357x over previous
"""LocalFeatureAggregation Trainium2 Bass kernel (v6: transfer+instruction optimized).

Reference computation (per batch b, point n):
  t[n,k,:]   = LeakyReLU_0.1(geom[n,k,:] @ w.T + b)          # [N,K,D], D=64
  fn[n,k,:]  = features[idx[n,k], :]                          # [N,K,C], C=64
  out[n,:]   = concat(mean_k t, mean_k fn)                    # [N, 128]

Empirical cost model of this deployment (measured):
  - host<->device tunnel ~65-75 MB/s; outputs cost double (donated zero
    buffers are uploaded, results downloaded)
  - ~33us per executed instruction, serial across engines per core
  - ~325us per indirect-DMA op (flat; one index per partition per op is
    the HW semantic), ~350us per matmul
So v6 minimizes bytes moved AND instruction count:
  - all float payloads bf16, indices int16 (N=16384 < 2^15); rel
    tolerance is 2e-2, bf16 adds <1e-2 worst case
  - core = (b = core//2, h = core%2) handles points h*8192..+8192 of
    batch b for both sides, full 64 channels (features[b] replicated
    per batch pair - cheaper than doubling the gather op count)
  - f-side: 1024 indirect gathers (one 128B row per partition, row
    n = n0 + 128q + p), CCE add folds the 16-neighbor sum into the DMA
  - t-side: geom uploaded pre-transposed; 128 matmuls with host-built
    block-stationary S_k bf16 -> ACT Prelu in place on PSUM -> DVE
    accumulate f32 -> ACT downcast; outputs stored in device layout,
    host reindexes.
"""

import sys

sys.path.insert(0, "/opt/trn_rl_repo")

import numpy as np
import ml_dtypes

import concourse.bass as bass
import concourse.tile as tile
from concourse import mybir
from concourse.bass_utils import run_bass_kernel_spmd

P = 128
B, N, K, C, D = 4, 16384, 16, 64, 64
NH = N // 2            # points per core
Q = NH // P            # f-side point groups per core (64)
T = NH * K * 4 // (P * P)  # 32
G = 8
W = T * P // G         # 512
F32 = mybir.dt.float32
BF = mybir.dt.bfloat16
I32 = mybir.dt.int32
I16 = mybir.dt.int16
BF_NP = np.dtype(ml_dtypes.bfloat16)

_CACHE = {}


class _SplitDrainTC(tile.TileContext):
    """TileContext whose tail drain splits its sem waits across multiple
    single-wait drain instructions (walrus accepts one sync-wait per
    instruction on this path)."""

    def _drain_and_barrier(self, tick_clock, wait_clock):
        from concourse.vector_clock import ScopedClock

        drain_inst = self.nc.sync.drain()
        wait_clock.add_sem_waits(
            drain_inst.ins, ScopedClock({None: tick_clock.global_clock})
        )
        inst = drain_inst.ins
        si = inst.sync_info
        waits = list(si.on_wait) if si else []
        if len(waits) > 1:
            si.on_wait = waits[:1]
            for w in waits[1:]:
                d2 = self.nc.sync.drain().ins
                if d2.sync_info is None:
                    d2.sync_info = mybir.SyncInfo(on_wait=[w], on_update=[])
                else:
                    d2.sync_info.on_wait = [w]
        self.nc.all_engine_barrier()
        popped = self.nc._tile_sem_poison_stack.pop()
        assert popped is self._sem_poison
        self.nc.clear_and_free_semaphores(list(self.sems.allocated().values()))
        self.nc.all_engine_barrier()


def _build_program(_variant="full"):
    nc = bass.Bass(
        "TRN2",
        target_bir_lowering=False,
        debug=False,
        enable_asserts=False,
        num_devices=8,
    )
    fd = nc.dram_tensor("fh", [N, C], BF, kind="ExternalInput")
    gt = nc.dram_tensor("gt", [P, T * P], BF, kind="ExternalInput")
    ixd = nc.dram_tensor("ix", [P, Q * K], I16, kind="ExternalInput")
    sd = nc.dram_tensor("s", [P, K * P], BF, kind="ExternalInput")
    bd = nc.dram_tensor("bias", [P, 2], F32, kind="ExternalInput")
    ot_d = nc.dram_tensor("ot", [P, T * P], BF, kind="ExternalOutput")
    of_d = nc.dram_tensor("of", [P, Q * C], BF, kind="ExternalOutput")

    from contextlib import ExitStack

    with _SplitDrainTC(nc) as tc, ExitStack() as ctx:
        const = ctx.enter_context(tc.tile_pool(name="const", bufs=1))
        big = ctx.enter_context(tc.tile_pool(name="big", bufs=1))
        tmp = ctx.enter_context(tc.tile_pool(name="tmp", bufs=2))
        ps1 = ctx.enter_context(tc.tile_pool(name="ps1", bufs=3, space="PSUM"))
        psw = ctx.enter_context(tc.tile_pool(name="psw", bufs=1, space="PSUM"))

        s_sb = const.tile([P, K * P], BF)
        nc.sync.dma_start(s_sb[:], sd.ap())
        b_sb = const.tile([P, 2], F32)
        nc.sync.dma_start(b_sb[:], bd.ap())
        ix16 = const.tile([P, Q * K], I16)
        nc.sync.dma_start(ix16[:], ixd.ap())
        bt = big.tile([P, T * P], BF)
        nc.sync.dma_start(bt[:], gt.ap())

        # Index upcast (the SWDGE index path wants i32); also absorbs the
        # ix DMA lane into the DVE clock.
        ix32 = const.tile([P, Q * K], I32)
        nc.vector.tensor_copy(ix32[:], ix16[:])

        # Warm-up observer ops: absorb each input-load DMA lane into the
        # engine that will consume that tensor, keeping every later
        # instruction at <=1 sync wait (walrus limit).
        warm_sb = tmp.tile([P, 1], F32)
        nc.scalar.activation(
            warm_sb[:], b_sb[:, 0:1], mybir.ActivationFunctionType.Copy,
            bias=0.0, scale=1.0,
        )
        wp = psw.tile([P, 2], F32)
        nc.tensor.matmul(
            out=wp[:, 0:1], lhsT=s_sb[:, 0:P], rhs=s_sb[:, 0:1],
            start=True, stop=True,
        )
        nc.tensor.matmul(
            out=wp[:, 1:2], lhsT=bt[:, 0:P], rhs=bt[:, 0:1],
            start=True, stop=True,
        )

        # -------- f-side: per-row indirect gathers, K-mean in the DMA ----
        facc = big.tile([P, Q * C], BF)  # [p, (q, c)]; n = n0 + 128*q + p
        if _variant == "nogather":
            nc.vector.memset(facc[:], 0.0)
        else:
            for q in range(Q):
                for k in range(K):
                    nc.gpsimd.indirect_dma_start(
                        out=facc[:, bass.ts(q, C)],
                        out_offset=None,
                        in_=fd.ap(),
                        in_offset=bass.IndirectOffsetOnAxis(
                            ap=ix32[:, q * K + k : q * K + k + 1], axis=0
                        ),
                        compute_op=(
                            mybir.AluOpType.add if k else mybir.AluOpType.bypass
                        ),
                    )
            nc.scalar.activation(
                facc[:], facc[:], mybir.ActivationFunctionType.Copy,
                bias=0.0, scale=1.0 / K,
            )
        nc.sync.dma_start(of_d.ap(), facc[:])

        # ---------------- t-side ----------------------------------------
        acc = big.tile([P, T * P], F32)
        ot_sb = big.tile([P, T * P], BF)
        for tg in range(G):
            for j in range(K):
                ps = ps1.tile([P, W], F32, tag="ps")
                nc.tensor.matmul(
                    out=ps[:],
                    lhsT=s_sb[:, bass.ts(j, P)],
                    rhs=bt[:, bass.ts(tg, W)],
                    start=True,
                    stop=True,
                )
                nc.scalar.activation(
                    ps[:],
                    ps[:],
                    mybir.ActivationFunctionType.Prelu,
                    bias=b_sb[:, 0:1],
                    scale=1.0 / K,
                    alpha=b_sb[:, 1:2],
                )
                if j == 0:
                    nc.vector.tensor_copy(acc[:, bass.ts(tg, W)], ps[:])
                else:
                    nc.vector.tensor_add(
                        acc[:, bass.ts(tg, W)], acc[:, bass.ts(tg, W)], ps[:]
                    )
            nc.scalar.activation(
                ot_sb[:, bass.ts(tg, W)],
                acc[:, bass.ts(tg, W)],
                mybir.ActivationFunctionType.Copy,
                bias=0.0,
                scale=1.0,
            )
        nc.sync.dma_start(ot_d.ap(), ot_sb[:])

    # ---- post passes: enforce <=1 sync wait per instruction -------------
    _ENGINE_SEM = {
        mybir.EngineType.PE: "PE_",
        mybir.EngineType.Activation: "Activation_",
        mybir.EngineType.DVE: "DVE_",
    }
    for inst in nc.inst_map.values():
        si = inst.sync_info
        if si is None or len(si.on_wait) <= 1:
            continue
        pref = _ENGINE_SEM.get(inst.engine)
        if pref is None:
            continue
        keep = [w for w in si.on_wait if not w.ant_name.startswith(pref)]
        if len(keep) < len(si.on_wait) and len(keep) <= 1:
            si.on_wait = keep

    # ACT waits transitively implied by the producing matmul's own waits
    # (same sem, >= threshold): strip them.
    last_mm = {}
    for inst in nc.inst_map.values():
        si = inst.sync_info
        if isinstance(inst, mybir.InstMatmult):
            for w in si.on_wait if si else []:
                last_mm[w.ant_name] = max(w.wait_value, last_mm.get(w.ant_name, 0))
        if (
            inst.engine == mybir.EngineType.Activation
            and si is not None
            and len(si.on_wait) > 1
        ):
            pe = [w for w in si.on_wait if w.ant_name.startswith("PE_")]
            rest = [w for w in si.on_wait if not w.ant_name.startswith("PE_")]
            if len(pe) == 1 and all(
                last_mm.get(w.ant_name, -1) >= w.wait_value for w in rest
            ):
                si.on_wait = pe

    # The chained accumulating gathers issue on one SWDGE FIFO and each
    # partition's descriptors drain on a fixed SDMA engine in order, so
    # cross-lane WAW completion waits between them are redundant.
    for inst in nc.inst_map.values():
        if not isinstance(inst, mybir.InstDMACopy):
            continue
        if getattr(inst, "queue", "") != "qPoolDynamic":
            continue
        si = inst.sync_info
        if si is None or len(si.on_wait) <= 1:
            continue
        non_sw = [w for w in si.on_wait if not w.ant_name.startswith("DMASW")]
        sw = [w for w in si.on_wait if w.ant_name.startswith("DMASW")]
        keep = non_sw if non_sw else sw[:1]
        if len(keep) == 1:
            si.on_wait = keep

    # Any instruction still waiting several SWDGE lanes: the gathers issue
    # on one FIFO and each SDMA engine drains its ring in order, so the
    # last lane's completion implies the earlier ones. Keep the last.
    for inst in nc.inst_map.values():
        si = inst.sync_info
        if si is None or len(si.on_wait) <= 1:
            continue
        sw = [w for w in si.on_wait if w.ant_name.startswith("DMASW")]
        if len(sw) == len(si.on_wait):
            si.on_wait = sw[-1:]

    # Output stores: keep the single compute-producer wait.
    for inst in nc.inst_map.values():
        if not isinstance(inst, mybir.InstDMACopy):
            continue
        si = inst.sync_info
        if si is None or len(si.on_wait) <= 1:
            continue
        memrefs = {getattr(a, "memref", "") for a in inst.outs}
        if memrefs <= {"ot", "of"}:
            act = [w for w in si.on_wait if w.ant_name.startswith("Activation_")]
            sw = [w for w in si.on_wait if w.ant_name.startswith("DMASW")]
            if len(act) == 1:
                si.on_wait = act
            elif len(sw) >= 1:
                si.on_wait = sw[-1:]
    return nc


def _host_inputs(features, geom, w, bvec, nbr):
    """Build the 8 per-core input dicts (pure layout prep, bf16/i16)."""
    S = np.zeros((P, K, P), np.float32)
    wT = np.ascontiguousarray(w.T)  # [4, 64]
    for j in range(K):
        for n2 in range(2):
            S[64 * n2 + 4 * j : 64 * n2 + 4 * j + 4, j, 64 * n2 : 64 * n2 + 64] = wT
    s_host = np.ascontiguousarray(S.reshape(P, K * P)).astype(BF_NP)
    bias_host = np.zeros((P, 2), np.float32)
    bias_host[:, 0] = np.tile(bvec / K, 2)
    bias_host[:, 1] = 0.1

    in_maps = []
    for core in range(8):
        b, h = divmod(core, 2)
        n0 = h * NH
        # bt layout: partition 64*n2 + 4*k + f, col 128*t + r;
        # point n = n0 + 256*t + 2*r + n2
        gt_host = np.ascontiguousarray(
            geom[b, n0 : n0 + NH]
            .reshape(T, P, 2, K, 4)
            .transpose(2, 3, 4, 0, 1)
            .reshape(P, T * P)
        ).astype(BF_NP)
        fh_host = np.ascontiguousarray(features[b]).astype(BF_NP)
        # ix[p, q*K+k] = nbr[b][n0 + 128*q + p, k]
        ix_host = np.ascontiguousarray(
            nbr[b, n0 : n0 + NH].reshape(Q, P, K).transpose(1, 0, 2).reshape(P, Q * K)
        ).astype(np.int16)
        in_maps.append(
            {
                "fh": fh_host,
                "gt": gt_host,
                "ix": ix_host,
                "s": s_host,
                "bias": bias_host,
            }
        )
    return in_maps


def kernel(**inputs):
    features = np.asarray(inputs["features"], np.float32)
    geom = np.asarray(inputs["geom_features"], np.float32)
    w = np.asarray(inputs["w"], np.float32)
    bvec = np.asarray(inputs["b"], np.float32)
    nbr = np.asarray(inputs["neighbor_indices"])

    if "nc" not in _CACHE:
        _CACHE["nc"] = _build_program()
    nc = _CACHE["nc"]

    in_maps = _host_inputs(features, geom, w, bvec, nbr)
    res = run_bass_kernel_spmd(nc, in_maps, list(range(8)))

    out = np.empty((B, N, 2 * D), np.float32)
    for core in range(8):
        b, h = divmod(core, 2)
        n0 = h * NH
        ot = np.asarray(res.results[core]["ot"]).astype(np.float32)  # [128, T*P]
        out[b, n0 : n0 + NH, :D] = (
            ot.reshape(2, D, T, P).transpose(2, 3, 0, 1).reshape(NH, D)
        )
        of = np.asarray(res.results[core]["of"]).astype(np.float32)  # [128, Q*C]
        out[b, n0 : n0 + NH, D:] = (
            of.reshape(P, Q, C).transpose(1, 0, 2).reshape(NH, C)
        )
    return out


# revision 28
# speedup vs baseline: 4.2386x; 1.0249x over previous
"""LocalFeatureAggregation Trainium2 Bass kernel (v6: transfer+instruction optimized).

Reference computation (per batch b, point n):
  t[n,k,:]   = LeakyReLU_0.1(geom[n,k,:] @ w.T + b)          # [N,K,D], D=64
  fn[n,k,:]  = features[idx[n,k], :]                          # [N,K,C], C=64
  out[n,:]   = concat(mean_k t, mean_k fn)                    # [N, 128]

Empirical cost model of this deployment (measured):
  - host<->device tunnel ~65-75 MB/s; outputs cost double (donated zero
    buffers are uploaded, results downloaded)
  - ~33us per executed instruction, serial across engines per core
  - ~325us per indirect-DMA op (flat; one index per partition per op is
    the HW semantic), ~350us per matmul
So v6 minimizes bytes moved AND instruction count:
  - all float payloads bf16, indices int16 (N=16384 < 2^15); rel
    tolerance is 2e-2, bf16 adds <1e-2 worst case
  - core = (b = core//2, h = core%2) handles points h*8192..+8192 of
    batch b for both sides, full 64 channels (features[b] replicated
    per batch pair - cheaper than doubling the gather op count)
  - f-side: 1024 indirect gathers (one 128B row per partition, row
    n = n0 + 128q + p), CCE add folds the 16-neighbor sum into the DMA
  - t-side: geom uploaded pre-transposed; 128 matmuls with host-built
    block-stationary S_k bf16 -> ACT Prelu in place on PSUM -> DVE
    accumulate f32 -> ACT downcast; outputs stored in device layout,
    host reindexes.
"""

import sys

sys.path.insert(0, "/opt/trn_rl_repo")

import numpy as np
import ml_dtypes

import concourse.bass as bass
import concourse.tile as tile
from concourse import mybir
from concourse.bass_utils import run_bass_kernel_spmd

P = 128
B, N, K, C, D = 4, 16384, 16, 64, 64
NH = N // 2            # points per core
Q = NH // P            # f-side point groups per core (64)
T = NH * K * 4 // (P * P)  # 32
G = 8
W = T * P // G         # 512
F32 = mybir.dt.float32
BF = mybir.dt.bfloat16
I32 = mybir.dt.int32
I16 = mybir.dt.int16
BF_NP = np.dtype(ml_dtypes.bfloat16)

_CACHE = {}


class _SplitDrainTC(tile.TileContext):
    """TileContext whose tail drain splits its sem waits across multiple
    single-wait drain instructions (walrus accepts one sync-wait per
    instruction on this path)."""

    def _drain_and_barrier(self, tick_clock, wait_clock):
        from concourse.vector_clock import ScopedClock

        drain_inst = self.nc.sync.drain()
        wait_clock.add_sem_waits(
            drain_inst.ins, ScopedClock({None: tick_clock.global_clock})
        )
        inst = drain_inst.ins
        si = inst.sync_info
        waits = list(si.on_wait) if si else []
        if len(waits) > 1:
            si.on_wait = waits[:1]
            for w in waits[1:]:
                d2 = self.nc.sync.drain().ins
                if d2.sync_info is None:
                    d2.sync_info = mybir.SyncInfo(on_wait=[w], on_update=[])
                else:
                    d2.sync_info.on_wait = [w]
        self.nc.all_engine_barrier()
        popped = self.nc._tile_sem_poison_stack.pop()
        assert popped is self._sem_poison
        self.nc.clear_and_free_semaphores(list(self.sems.allocated().values()))
        self.nc.all_engine_barrier()


def _build_program(_variant="full"):
    nc = bass.Bass(
        "TRN2",
        target_bir_lowering=False,
        debug=False,
        enable_asserts=False,
        num_devices=8,
    )
    fd = nc.dram_tensor("fh", [N, C], BF, kind="ExternalInput")
    gt = nc.dram_tensor("gt", [P, T * P], BF, kind="ExternalInput")
    ixd = nc.dram_tensor("ix", [P, Q * K], I16, kind="ExternalInput")
    # s_big[64*kg + 8*km + 4*n2 + f, 128*km' + 64*n2' + d] = w[d, f] when
    # km==km' and n2==n2', else 0. The per-k stationary is the 64-partition
    # slice at base 64*(k//8) (legal PE base), column block k%8; its zero
    # rows mask out the other seven k's sharing the partition group.
    sd = nc.dram_tensor("s", [P, 8 * P], BF, kind="ExternalInput")
    bd = nc.dram_tensor("bias", [P, 2], F32, kind="ExternalInput")
    ot_d = nc.dram_tensor("ot", [P, T * P], BF, kind="ExternalOutput")
    of_d = nc.dram_tensor("of", [P, Q * C], BF, kind="ExternalOutput")

    from contextlib import ExitStack

    with _SplitDrainTC(nc) as tc, ExitStack() as ctx:
        const = ctx.enter_context(tc.tile_pool(name="const", bufs=1))
        big = ctx.enter_context(tc.tile_pool(name="big", bufs=1))
        tmp = ctx.enter_context(tc.tile_pool(name="tmp", bufs=2))
        ps1 = ctx.enter_context(tc.tile_pool(name="ps1", bufs=2, space="PSUM"))

        s_sb = const.tile([P, 8 * P], BF)
        nc.sync.dma_start(s_sb[:], sd.ap())
        b_sb = const.tile([P, 2], F32)
        nc.sync.dma_start(b_sb[:], bd.ap())
        ix16 = const.tile([P, Q * K], I16)
        nc.sync.dma_start(ix16[:], ixd.ap())
        bt = big.tile([P, T * P], BF)
        nc.sync.dma_start(bt[:], gt.ap())

        # Index upcast (the SWDGE index path wants i32); also absorbs the
        # ix DMA lane into the DVE clock.
        ix32 = const.tile([P, Q * K], I32)
        nc.vector.tensor_copy(ix32[:], ix16[:])

        # Warm-up observer ops: absorb each input-load DMA lane into the
        # engine that will consume that tensor, keeping every later
        # instruction at <=1 sync wait (walrus limit).
        warm_sb = tmp.tile([P, 1], F32)
        nc.scalar.activation(
            warm_sb[:], b_sb[:, 0:1], mybir.ActivationFunctionType.Copy,
            bias=0.0, scale=1.0,
        )
        wp = ps1.tile([P, 4 * W], F32, tag="ps")
        nc.tensor.matmul(
            out=wp[:, 0:1], lhsT=s_sb[:, 0:P], rhs=s_sb[:, 0:1],
            start=True, stop=True,
        )
        nc.tensor.matmul(
            out=wp[:, 1:2], lhsT=bt[:, 0:P], rhs=bt[:, 0:1],
            start=True, stop=True,
        )

        # -------- f-side: per-row indirect gathers, K-mean in the DMA ----
        facc = big.tile([P, Q * C], BF)  # [p, (q, c)]; n = n0 + 128*q + p
        if _variant == "nogather":
            nc.vector.memset(facc[:], 0.0)
        else:
            for q in range(Q):
                for k in range(K):
                    nc.gpsimd.indirect_dma_start(
                        out=facc[:, bass.ts(q, C)],
                        out_offset=None,
                        in_=fd.ap(),
                        in_offset=bass.IndirectOffsetOnAxis(
                            ap=ix32[:, q * K + k : q * K + k + 1], axis=0
                        ),
                        compute_op=(
                            mybir.AluOpType.add if k else mybir.AluOpType.bypass
                        ),
                    )
            nc.scalar.activation(
                facc[:], facc[:], mybir.ActivationFunctionType.Copy,
                bias=0.0, scale=1.0 / K,
            )
        nc.sync.dma_start(of_d.ap(), facc[:])

        # ---------------- t-side ----------------------------------------
        # Per k: an 8-partition slice (n2, f at 4k) of s_c is the stationary
        # and the matching partition slice of bt is the moving data. 4
        # matmuls of 512 cols fill one 4-bank [128, 2048] PSUM tile; then a
        # single Prelu (in place) and a single DVE accumulate per span.
        W2 = 4 * W  # 2048
        G2 = T * P // W2  # 2
        acc = big.tile([P, T * P], F32)
        ot_sb = big.tile([P, T * P], BF)
        for tg in range(G2):
            for j in range(K):
                ps = ps1.tile([P, W2], F32, tag="ps")
                kg, km = divmod(j, 8)
                for sb in range(4):
                    nc.tensor.matmul(
                        out=ps[:, bass.ts(sb, W)],
                        lhsT=s_sb[64 * kg : 64 * kg + 64, bass.ts(km, P)],
                        rhs=bt[
                            64 * kg : 64 * kg + 64,
                            tg * W2 + sb * W : tg * W2 + (sb + 1) * W,
                        ],
                        start=True,
                        stop=True,
                    )
                nc.scalar.activation(
                    ps[:],
                    ps[:],
                    mybir.ActivationFunctionType.Prelu,
                    bias=b_sb[:, 0:1],
                    scale=1.0 / K,
                    alpha=b_sb[:, 1:2],
                )
                if j == 0:
                    nc.vector.tensor_copy(acc[:, bass.ts(tg, W2)], ps[:])
                else:
                    nc.vector.tensor_add(
                        acc[:, bass.ts(tg, W2)], acc[:, bass.ts(tg, W2)], ps[:]
                    )
            nc.scalar.activation(
                ot_sb[:, bass.ts(tg, W2)],
                acc[:, bass.ts(tg, W2)],
                mybir.ActivationFunctionType.Copy,
                bias=0.0,
                scale=1.0,
            )
        nc.sync.dma_start(ot_d.ap(), ot_sb[:])

    # ---- post passes: enforce <=1 sync wait per instruction -------------
    _ENGINE_SEM = {
        mybir.EngineType.PE: "PE_",
        mybir.EngineType.Activation: "Activation_",
        mybir.EngineType.DVE: "DVE_",
    }
    for inst in nc.inst_map.values():
        si = inst.sync_info
        if si is None or len(si.on_wait) <= 1:
            continue
        pref = _ENGINE_SEM.get(inst.engine)
        if pref is None:
            continue
        keep = [w for w in si.on_wait if not w.ant_name.startswith(pref)]
        if len(keep) < len(si.on_wait) and len(keep) <= 1:
            si.on_wait = keep

    # ACT waits transitively implied by the producing matmul's own waits
    # (same sem, >= threshold): strip them.
    last_mm = {}
    for inst in nc.inst_map.values():
        si = inst.sync_info
        if isinstance(inst, mybir.InstMatmult):
            for w in si.on_wait if si else []:
                last_mm[w.ant_name] = max(w.wait_value, last_mm.get(w.ant_name, 0))
        if (
            inst.engine == mybir.EngineType.Activation
            and si is not None
            and len(si.on_wait) > 1
        ):
            pe = [w for w in si.on_wait if w.ant_name.startswith("PE_")]
            rest = [w for w in si.on_wait if not w.ant_name.startswith("PE_")]
            if len(pe) == 1 and all(
                last_mm.get(w.ant_name, -1) >= w.wait_value for w in rest
            ):
                si.on_wait = pe

    # DVE accumulates wait on the Prelu (ACT) plus the PSUM-writing matmuls
    # (PE) plus their own engine. The Prelu already waited on those same
    # matmuls, so the ACT wait implies the PE wait; own-engine waits are
    # implied by queue order. Verify coverage and strip.
    last_act_pe = 0
    for inst in nc.inst_map.values():
        si = inst.sync_info
        if inst.engine == mybir.EngineType.Activation:
            for w in si.on_wait if si else []:
                if w.ant_name.startswith("PE_"):
                    last_act_pe = max(last_act_pe, w.wait_value)
        if (
            inst.engine == mybir.EngineType.DVE
            and si is not None
            and len(si.on_wait) > 1
        ):
            act = [w for w in si.on_wait if w.ant_name.startswith("Activation_")]
            others = [w for w in si.on_wait if not w.ant_name.startswith("Activation_")]
            if len(act) == 1 and all(
                w.ant_name.startswith("DVE_")
                or (w.ant_name.startswith("PE_") and w.wait_value <= last_act_pe)
                for w in others
            ):
                si.on_wait = act

    # The chained accumulating gathers issue on one SWDGE FIFO and each
    # partition's descriptors drain on a fixed SDMA engine in order, so
    # cross-lane WAW completion waits between them are redundant.
    for inst in nc.inst_map.values():
        if not isinstance(inst, mybir.InstDMACopy):
            continue
        if getattr(inst, "queue", "") != "qPoolDynamic":
            continue
        si = inst.sync_info
        if si is None or len(si.on_wait) <= 1:
            continue
        non_sw = [w for w in si.on_wait if not w.ant_name.startswith("DMASW")]
        sw = [w for w in si.on_wait if w.ant_name.startswith("DMASW")]
        keep = non_sw if non_sw else sw[:1]
        if len(keep) == 1:
            si.on_wait = keep

    # Any instruction still waiting several SWDGE lanes: the gathers issue
    # on one FIFO and each SDMA engine drains its ring in order, so the
    # last lane's completion implies the earlier ones. Keep the last.
    for inst in nc.inst_map.values():
        si = inst.sync_info
        if si is None or len(si.on_wait) <= 1:
            continue
        sw = [w for w in si.on_wait if w.ant_name.startswith("DMASW")]
        if len(sw) == len(si.on_wait):
            si.on_wait = sw[-1:]

    # Output stores: keep the single compute-producer wait.
    for inst in nc.inst_map.values():
        if not isinstance(inst, mybir.InstDMACopy):
            continue
        si = inst.sync_info
        if si is None or len(si.on_wait) <= 1:
            continue
        memrefs = {getattr(a, "memref", "") for a in inst.outs}
        if memrefs <= {"ot", "of"}:
            act = [w for w in si.on_wait if w.ant_name.startswith("Activation_")]
            sw = [w for w in si.on_wait if w.ant_name.startswith("DMASW")]
            if len(act) == 1:
                si.on_wait = act
            elif len(sw) >= 1:
                si.on_wait = sw[-1:]
    return nc


def _host_inputs(features, geom, w, bvec, nbr):
    """Build the 8 per-core input dicts (pure layout prep, bf16/i16)."""
    S = np.zeros((P, 8 * P), np.float32)
    wT = np.ascontiguousarray(w.T)  # [4, 64]
    for j in range(K):
        kg, km = divmod(j, 8)
        for n2 in range(2):
            r0 = 64 * kg + 8 * km + 4 * n2
            S[r0 : r0 + 4, 128 * km + 64 * n2 : 128 * km + 64 * n2 + 64] = wT
    s_host = np.ascontiguousarray(S).astype(BF_NP)
    bias_host = np.zeros((P, 2), np.float32)
    bias_host[:, 0] = np.tile(bvec / K, 2)
    bias_host[:, 1] = 0.1

    in_maps = []
    for core in range(8):
        b, h = divmod(core, 2)
        n0 = h * NH
        # bt layout: partition 8*k + 4*n2 + f, col 128*t + r;
        # point n = n0 + 256*t + 2*r + n2
        gt_host = np.ascontiguousarray(
            geom[b, n0 : n0 + NH]
            .reshape(T, P, 2, K, 4)
            .transpose(3, 2, 4, 0, 1)
            .reshape(P, T * P)
        ).astype(BF_NP)
        fh_host = np.ascontiguousarray(features[b]).astype(BF_NP)
        # ix[p, q*K+k] = nbr[b][n0 + 128*q + p, k]
        ix_host = np.ascontiguousarray(
            nbr[b, n0 : n0 + NH].reshape(Q, P, K).transpose(1, 0, 2).reshape(P, Q * K)
        ).astype(np.int16)
        in_maps.append(
            {
                "fh": fh_host,
                "gt": gt_host,
                "ix": ix_host,
                "s": s_host,
                "bias": bias_host,
            }
        )
    return in_maps


def kernel(**inputs):
    features = np.asarray(inputs["features"], np.float32)
    geom = np.asarray(inputs["geom_features"], np.float32)
    w = np.asarray(inputs["w"], np.float32)
    bvec = np.asarray(inputs["b"], np.float32)
    nbr = np.asarray(inputs["neighbor_indices"])

    if "nc" not in _CACHE:
        _CACHE["nc"] = _build_program()
    nc = _CACHE["nc"]

    in_maps = _host_inputs(features, geom, w, bvec, nbr)
    res = run_bass_kernel_spmd(nc, in_maps, list(range(8)))

    out = np.empty((B, N, 2 * D), np.float32)
    for core in range(8):
        b, h = divmod(core, 2)
        n0 = h * NH
        ot = np.asarray(res.results[core]["ot"]).astype(np.float32)  # [128, T*P]
        out[b, n0 : n0 + NH, :D] = (
            ot.reshape(2, D, T, P).transpose(2, 3, 0, 1).reshape(NH, D)
        )
        of = np.asarray(res.results[core]["of"]).astype(np.float32)  # [128, Q*C]
        out[b, n0 : n0 + NH, D:] = (
            of.reshape(P, Q, C).transpose(1, 0, 2).reshape(NH, C)
        )
    return out
